# revision 15
# baseline (speedup 1.0000x reference)
"""MoE (top-2 routing, 16 experts, silu MLP) on 8 Trainium2 NeuronCores.

Strategy (expert parallelism):
  - Host: router (x @ w_router + b_router, top-2, softmax), token dispatch.
  - Each core owns 2 expert "slots". Experts are ranked by routed-token
    count: the 8 largest go in slot 0 (static capacity C0 = largest count),
    the 8 smallest in slot 1 (C1). Tokens for an expert are gathered,
    transposed to [D, n_tokens], zero-padded to the slot capacity and
    shipped to the owning core as bf16.
  - Device (SPMD, one Bass program on all 8 cores), per slot:
        h = silu(W1.T @ xT + b1)   [U, C]   (bf16 matmuls, fp32 PSUM)
        y = W2.T @ h               [D, C]
    W1 streams from HBM per 128-row block, xT chunks live in SBUF, h
    bounces through an internal DRAM buffer, and each slot's W2 is staged
    into SBUF ahead of use (slot 0 during FFN1, slot 1 during slot-0 FFN2)
    so the PE never stalls on weights at phase edges. Activation stores go
    out on the gpsimd DMA queue so they never block the load queue.
  - Host: out = x + sum_e combine_weight_e * (y_e + b2_e) scatter-added.

The kernel is self-contained: it hardcodes the model shapes and builds /
compiles / runs the Bass program at call time via run_bass_kernel_spmd.
"""

import sys
import types

import ml_dtypes
import numpy as np

B, D, E, U, TOPK = 16384, 1024, 16, 4096, 2
N_CORES = 8
S = E // N_CORES  # expert slots per core = 2
P = 128
KO1 = D // P      # 8   k-tiles for layer 1
MO1 = U // P      # 32  m-tiles for layer 1
KO2 = U // P      # 32  k-tiles for layer 2
MO2 = D // P      # 8   m-tiles for layer 2
NCHUNK = 512      # token-column chunk for layer 1 (matmul moving free dim)
NCHUNK2 = 384     # token-column chunk for layer 2 (smaller: SBUF budget)

BF16 = ml_dtypes.bfloat16

# Stash of the most recent BassKernelResults (test harness reads timing here).
LAST_RESULTS = None
TRACE = False
TRACE_CORES = None


def _install_ntff_hook_shim():
    """Make `antenv.axon_hooks` importable so run_bass_kernel_spmd(trace=True)
    works; the image's antenv package lacks this optional module."""
    if "antenv.axon_hooks" in sys.modules:
        return
    try:
        import antenv.axon_hooks  # noqa: F401

        return
    except ImportError:
        pass
    try:
        import antenv
    except ImportError:
        return
    mod = types.ModuleType("antenv.axon_hooks")
    mod._hook = None

    def set_axon_ntff_profile_hook(h):
        mod._hook = h

    def get_axon_ntff_profile_hook():
        return mod._hook

    mod.set_axon_ntff_profile_hook = set_axon_ntff_profile_hook
    mod.get_axon_ntff_profile_hook = get_axon_ntff_profile_hook
    sys.modules["antenv.axon_hooks"] = mod
    antenv.axon_hooks = mod
    try:
        from trn_agent_boot.trn_boot import _ntff_profile_via_ctypes

        hook = _ntff_profile_via_ctypes("/opt/axon/libaxon_pjrt.so")
        if hook is not None:
            mod._hook = hook
    except Exception:
        pass


def _chunks(total, step=NCHUNK):
    out = []
    c = 0
    while c < total:
        out.append((c, min(step, total - c)))
        c += step
    return out


_PROGRAM_CACHE = {}


def _build_program(caps):
    """Build + compile the SPMD Bass program for per-slot capacities caps."""
    caps = tuple(int(c) for c in caps)
    if caps in _PROGRAM_CACHE:
        return _PROGRAM_CACHE[caps]

    import concourse.tile as tile
    from concourse import bacc, mybir
    from concourse.bass import ts

    f32 = mybir.dt.float32
    bf16 = mybir.dt.bfloat16
    Silu = mybir.ActivationFunctionType.Silu

    CT = sum(caps)
    soff = [0]
    for c in caps:
        soff.append(soff[-1] + c)

    nc = bacc.Bacc(None, target_bir_lowering=False, debug=False)
    xT = nc.dram_tensor("xT", [P, KO1, CT], bf16, kind="ExternalInput")
    w1s = nc.dram_tensor("w1s", [S, P, MO1, KO1, P], bf16, kind="ExternalInput")
    b1s = nc.dram_tensor("b1s", [S, P, MO1], f32, kind="ExternalInput")
    w2s = nc.dram_tensor("w2s", [S, P, KO2, D], bf16, kind="ExternalInput")
    yT = nc.dram_tensor("yT", [P, MO2, CT], f32, kind="ExternalOutput")

    with tile.TileContext(nc) as tc:
        with (
            tc.tile_pool(name="dram", bufs=S, space="DRAM") as dram,
            tc.tile_pool(name="bias", bufs=1) as biasp,
            tc.tile_pool(name="psum", bufs=6, space="PSUM") as psump,
            tc.tile_pool(name="w2a", bufs=1) as w2apool,
            tc.tile_pool(name="hin", bufs=2) as hipool,
            tc.tile_pool(name="yt", bufs=4) as ypool,
        ):
            # Slot-0 W2 gets dedicated space so the FFN1 -> FFN2 boundary
            # never stalls on a weight load; its DMA is emitted after the
            # slot-0 FFN1 code so it doesn't delay the startup x/w1 loads.
            w2sb0 = w2apool.tile([P, KO2, D], bf16, tag="w2a", name="w2sb0")

            b1_sb = biasp.tile([P, S, MO1], f32, tag="b1")
            for s in range(S):
                nc.sync.dma_start(b1_sb[:, s, :], b1s[s])

            hs = []
            # ---- layer 1: h = silu(W1.T @ xT + b1), per expert slot ----
            n_xbufs = sum(len(_chunks(c)) for c in caps)
            with (
                tc.tile_pool(name="xsb", bufs=min(n_xbufs, 8)) as xpool,
                tc.tile_pool(name="w1t", bufs=3) as w1pool,
                tc.tile_pool(name="hout", bufs=8) as hopool,
            ):
                for s in range(S):
                    if s == 1:
                        # load during slot-0 compute / slot-1 FFN1
                        nc.sync.dma_start(w2sb0[:], w2s[0])
                    cols = _chunks(caps[s])
                    xcs = []
                    for ci, (c0, w) in enumerate(cols):
                        xc = xpool.tile([P, KO1, NCHUNK], bf16, tag="xsb")
                        if s == 0 and ci == 0:
                            # First load is on the critical path: split it
                            # across k so the packets spread over DMA queues.
                            for k in range(KO1):
                                nc.sync.dma_start(
                                    xc[:, k, :w],
                                    xT[:, k, soff[s] + c0 : soff[s] + c0 + w],
                                )
                        else:
                            nc.sync.dma_start(
                                xc[:, :, :w], xT[:, :, soff[s] + c0 : soff[s] + c0 + w]
                            )
                        xcs.append(xc)
                    h_dram = dram.tile([P, MO1, caps[s]], bf16, tag="h")
                    for m in range(MO1):
                        wt = w1pool.tile([P, KO1, P], bf16, tag="w1t")
                        if s == 0 and m == 0:
                            for k in range(KO1):
                                nc.sync.dma_start(wt[:, k], w1s[s, :, m, k])
                        else:
                            nc.sync.dma_start(wt[:], w1s[s, :, m])
                        for ci, (c0, w) in enumerate(cols):
                            ps = psump.tile([P, NCHUNK], f32, tag="ps")
                            for k in range(KO1):
                                nc.tensor.matmul(
                                    ps[:, :w],
                                    wt[:, k],
                                    xcs[ci][:, k, :w],
                                    start=(k == 0),
                                    stop=(k == KO1 - 1),
                                )
                            ho = hopool.tile([P, NCHUNK], bf16, tag="ho")
                            nc.scalar.activation(
                                ho[:, :w], ps[:, :w], Silu, bias=b1_sb[:, s, m : m + 1]
                            )
                            nc.gpsimd.dma_start(h_dram[:, m, c0 : c0 + w], ho[:, :w])
                    hs.append(h_dram)

            # ---- layer 2: y = W2.T @ h, per expert slot ----
            # Slot-1 W2 loads into the space the FFN1 pools just freed,
            # overlapping slot-0 FFN2 compute.
            with tc.tile_pool(name="w2b", bufs=1) as w2bpool:
                w2sb1 = w2bpool.tile([P, KO2, D], bf16, tag="w2b", name="w2sb1")
                nc.sync.dma_start(w2sb1[:], w2s[1])
                for s in range(S):
                    w2sb = w2sb0 if s == 0 else w2sb1
                    cols = _chunks(caps[s], NCHUNK2)
                    for c0, w in cols:
                        hi = hipool.tile([P, KO2, NCHUNK2], bf16, tag="hi")
                        nc.sync.dma_start(hi[:, :, :w], hs[s][:, :, c0 : c0 + w])
                        for m2 in range(MO2):
                            ps = psump.tile([P, NCHUNK], f32, tag="ps")
                            for k2 in range(KO2):
                                nc.tensor.matmul(
                                    ps[:, :w],
                                    w2sb[:, k2, ts(m2, P)],
                                    hi[:, k2, :w],
                                    start=(k2 == 0),
                                    stop=(k2 == KO2 - 1),
                                )
                            yt = ypool.tile([P, NCHUNK2], f32, tag="yt")
                            nc.vector.tensor_copy(yt[:, :w], ps[:, :w])
                            nc.gpsimd.dma_start(
                                yT[:, m2, soff[s] + c0 : soff[s] + c0 + w],
                                yt[:, :w],
                            )

    nc.compile()
    _PROGRAM_CACHE[caps] = nc
    return nc


def kernel(x, w_router, b_router, w1, b1, w2, b2):
    _install_ntff_hook_shim()
    from concourse.bass_utils import run_bass_kernel_spmd

    x = np.asarray(x, dtype=np.float32)
    w_router = np.asarray(w_router, dtype=np.float32)
    b_router = np.asarray(b_router, dtype=np.float32)
    w1 = np.asarray(w1, dtype=np.float32)
    b1 = np.asarray(b1, dtype=np.float32)
    w2 = np.asarray(w2, dtype=np.float32)
    b2 = np.asarray(b2, dtype=np.float32)

    # ---- host router: top-2 + softmax over the selected logits ----
    logits = x @ w_router + b_router                    # [B, E]
    idx2 = np.argpartition(-logits, TOPK, axis=1)[:, :TOPK]
    vals = np.take_along_axis(logits, idx2, axis=1)
    order = np.argsort(-vals, axis=1)
    topk_i = np.take_along_axis(idx2, order, axis=1)    # [B, 2] descending
    topk_v = np.take_along_axis(vals, order, axis=1)
    topk_v = topk_v - topk_v.max(axis=1, keepdims=True)
    ew = np.exp(topk_v)
    cw = ew / ew.sum(axis=1, keepdims=True)             # [B, 2]

    # ---- dispatch: group (token, expert) pairs by expert ----
    eids = topk_i.ravel()                               # [B*2]
    toks = np.repeat(np.arange(B, dtype=np.int64), TOPK)
    wts = cw.ravel().astype(np.float32)
    perm = np.argsort(eids, kind="stable")
    toks_s, wts_s = toks[perm], wts[perm]
    counts = np.bincount(eids, minlength=E)
    offs = np.concatenate([[0], np.cumsum(counts)])

    # Rank experts by load: biggest 8 -> slot 0, smallest 8 -> slot 1.
    rank = np.argsort(-counts, kind="stable")           # expert ids, desc count
    # slot_expert[c][s] = expert id handled by core c, slot s
    slot_expert = [[int(rank[s * N_CORES + c]) for s in range(S)] for c in range(N_CORES)]
    caps = tuple(
        max(512, int(max(counts[rank[s * N_CORES + c]] for c in range(N_CORES))))
        for s in range(S)
    )
    CT = sum(caps)
    soff = [0]
    for c_ in caps:
        soff.append(soff[-1] + c_)

    nc = _build_program(caps)

    xTfull = np.ascontiguousarray(x.T)                  # [D, B] f32

    in_maps = []
    tok_lists = []
    for c in range(N_CORES):
        xcore = np.zeros((D, CT), dtype=BF16)
        core_toks = []
        for s in range(S):
            e = slot_expert[c][s]
            te = toks_s[offs[e] : offs[e + 1]]
            core_toks.append((te, wts_s[offs[e] : offs[e + 1]], e))
            xcore[:, soff[s] : soff[s] + len(te)] = xTfull[:, te].astype(BF16)
        tok_lists.append(core_toks)
        # [D, CT] -> [P, KO1, CT] with d = ko*P + p
        xcore = np.ascontiguousarray(xcore.reshape(KO1, P, CT).transpose(1, 0, 2))
        es = [slot_expert[c][s] for s in range(S)]
        w1c = np.ascontiguousarray(
            w1[es].astype(BF16).reshape(S, KO1, P, MO1, P).transpose(0, 2, 3, 1, 4)
        )  # [S, P, MO1, KO1, P]
        b1c = np.ascontiguousarray(
            b1[es].reshape(S, MO1, P).transpose(0, 2, 1)
        )  # [S, P, MO1]
        w2c = np.ascontiguousarray(
            w2[es].astype(BF16).reshape(S, KO2, P, D).transpose(0, 2, 1, 3)
        )  # [S, P, KO2, D]
        in_maps.append({"xT": xcore, "w1s": w1c, "b1s": b1c, "w2s": w2c})

    kw = {}
    if TRACE:
        kw = dict(trace=True)
        if TRACE_CORES is not None:
            kw["trace_cores"] = TRACE_CORES
    res = run_bass_kernel_spmd(nc, in_maps, core_ids=list(range(N_CORES)), **kw)
    global LAST_RESULTS
    LAST_RESULTS = res

    # ---- host combine: out = x + sum_e cw_e * (y_e + b2_e) ----
    out = x.copy()
    for c in range(N_CORES):
        yT = np.asarray(res.results[c]["yT"], dtype=np.float32)  # [P, MO2, CT]
        for s in range(S):
            te, we, e = tok_lists[c][s]
            n = len(te)
            if n == 0:
                continue
            y2 = yT[:, :, soff[s] : soff[s] + n]                 # [P, MO2, n]
            y2 = y2.transpose(1, 0, 2).reshape(D, n)             # d = m2*P + p
            out[te] += we[:, None] * (y2.T + b2[e])
    return out


# revision 17
# speedup vs baseline: 1.0213x; 1.0213x over previous
"""MoE (top-2 routing, 16 experts, silu MLP) on 8 Trainium2 NeuronCores.

Strategy (expert parallelism):
  - Host: router (x @ w_router + b_router, top-2, softmax), token dispatch.
  - Each core owns 2 expert "slots". Experts are ranked by routed-token
    count: the 8 largest go in slot 0 (static capacity C0 = largest count),
    the 8 smallest in slot 1 (C1). Tokens for an expert are gathered,
    transposed to [D, n_tokens], zero-padded to the slot capacity and
    shipped to the owning core as bf16.
  - Device (SPMD, one Bass program on all 8 cores), per slot:
        h = silu(W1.T @ xT + b1)   [U, C]   (bf16 matmuls, fp32 PSUM)
        y = W2.T @ h               [D, C]
    W1 streams from HBM per 128-row block, xT chunks live in SBUF, h
    bounces through an internal DRAM buffer, and each slot's W2 is staged
    into SBUF ahead of use (slot 0 during FFN1, slot 1 during slot-0 FFN2)
    so the PE never stalls on weights at phase edges. Activation stores go
    out on the gpsimd DMA queue so they never block the load queue.
  - Host: out = x + sum_e combine_weight_e * (y_e + b2_e) scatter-added.

The kernel is self-contained: it hardcodes the model shapes and builds /
compiles / runs the Bass program at call time via run_bass_kernel_spmd.
"""

import sys
import types

import ml_dtypes
import numpy as np

B, D, E, U, TOPK = 16384, 1024, 16, 4096, 2
N_CORES = 8
S = E // N_CORES  # expert slots per core = 2
P = 128
KO1 = D // P      # 8   k-tiles for layer 1
MO1 = U // P      # 32  m-tiles for layer 1
KO2 = U // P      # 32  k-tiles for layer 2
MO2 = D // P      # 8   m-tiles for layer 2
NCHUNK = 512      # token-column chunk for layer 1 (matmul moving free dim)
NCHUNK2 = 384     # token-column chunk for layer 2 (smaller: SBUF budget)

BF16 = ml_dtypes.bfloat16

# Stash of the most recent BassKernelResults (test harness reads timing here).
LAST_RESULTS = None
TRACE = False
TRACE_CORES = None


def _install_ntff_hook_shim():
    """Make `antenv.axon_hooks` importable so run_bass_kernel_spmd(trace=True)
    works; the image's antenv package lacks this optional module."""
    if "antenv.axon_hooks" in sys.modules:
        return
    try:
        import antenv.axon_hooks  # noqa: F401

        return
    except ImportError:
        pass
    try:
        import antenv
    except ImportError:
        return
    mod = types.ModuleType("antenv.axon_hooks")
    mod._hook = None

    def set_axon_ntff_profile_hook(h):
        mod._hook = h

    def get_axon_ntff_profile_hook():
        return mod._hook

    mod.set_axon_ntff_profile_hook = set_axon_ntff_profile_hook
    mod.get_axon_ntff_profile_hook = get_axon_ntff_profile_hook
    sys.modules["antenv.axon_hooks"] = mod
    antenv.axon_hooks = mod
    try:
        from trn_agent_boot.trn_boot import _ntff_profile_via_ctypes

        hook = _ntff_profile_via_ctypes("/opt/axon/libaxon_pjrt.so")
        if hook is not None:
            mod._hook = hook
    except Exception:
        pass


def _chunks(total, step=NCHUNK):
    out = []
    c = 0
    while c < total:
        out.append((c, min(step, total - c)))
        c += step
    return out


_PROGRAM_CACHE = {}


def _build_program(caps):
    """Build + compile the SPMD Bass program for per-slot capacities caps."""
    caps = tuple(int(c) for c in caps)
    if caps in _PROGRAM_CACHE:
        return _PROGRAM_CACHE[caps]

    import concourse.tile as tile
    from concourse import bacc, mybir
    from concourse.bass import ts

    f32 = mybir.dt.float32
    bf16 = mybir.dt.bfloat16
    Silu = mybir.ActivationFunctionType.Silu

    CT = sum(caps)
    soff = [0]
    for c in caps:
        soff.append(soff[-1] + c)

    nc = bacc.Bacc(None, target_bir_lowering=False, debug=False)
    xT = nc.dram_tensor("xT", [P, KO1, CT], bf16, kind="ExternalInput")
    w1s = nc.dram_tensor("w1s", [S, P, MO1, KO1, P], bf16, kind="ExternalInput")
    b1s = nc.dram_tensor("b1s", [S, P, MO1], f32, kind="ExternalInput")
    w2s = nc.dram_tensor("w2s", [S, P, KO2, D], bf16, kind="ExternalInput")
    yT = nc.dram_tensor("yT", [P, MO2, CT], f32, kind="ExternalOutput")

    with tile.TileContext(nc) as tc:
        with (
            tc.tile_pool(name="dram", bufs=S, space="DRAM") as dram,
            tc.tile_pool(name="bias", bufs=1) as biasp,
            tc.tile_pool(name="psum", bufs=6, space="PSUM") as psump,
            tc.tile_pool(name="w2a", bufs=1) as w2apool,
            tc.tile_pool(name="hin", bufs=2) as hipool,
            tc.tile_pool(name="yt", bufs=4) as ypool,
        ):
            # Slot-0 W2 gets dedicated space so the FFN1 -> FFN2 boundary
            # never stalls on a weight load; its DMA is emitted after the
            # slot-0 FFN1 code so it doesn't delay the startup x/w1 loads.
            w2sb0 = w2apool.tile([P, KO2, D], bf16, tag="w2a", name="w2sb0")

            b1_sb = biasp.tile([P, S, MO1], f32, tag="b1")
            for s in range(S):
                nc.sync.dma_start(b1_sb[:, s, :], b1s[s])

            hs = []
            # ---- layer 1: h = silu(W1.T @ xT + b1), per expert slot ----
            n_xbufs = sum(len(_chunks(c)) for c in caps)
            with (
                tc.tile_pool(name="xsb", bufs=min(n_xbufs, 8)) as xpool,
                tc.tile_pool(name="w1t", bufs=3) as w1pool,
                tc.tile_pool(name="hout", bufs=8) as hopool,
            ):
                for s in range(S):
                    if s == 1:
                        # load during slot-0 compute / slot-1 FFN1
                        nc.sync.dma_start(w2sb0[:], w2s[0])
                    cols = _chunks(caps[s])
                    xcs = []
                    for ci, (c0, w) in enumerate(cols):
                        xc = xpool.tile([P, KO1, NCHUNK], bf16, tag="xsb")
                        if s == 0 and ci == 0:
                            # First load is on the critical path: split it
                            # across k so the packets spread over DMA queues.
                            for k in range(KO1):
                                nc.sync.dma_start(
                                    xc[:, k, :w],
                                    xT[:, k, soff[s] + c0 : soff[s] + c0 + w],
                                )
                        else:
                            nc.sync.dma_start(
                                xc[:, :, :w], xT[:, :, soff[s] + c0 : soff[s] + c0 + w]
                            )
                        xcs.append(xc)
                    h_dram = dram.tile([P, MO1, caps[s]], bf16, tag="h")
                    for m in range(MO1):
                        wt = w1pool.tile([P, KO1, P], bf16, tag="w1t")
                        if s == 0 and m == 0:
                            for k in range(KO1):
                                nc.scalar.dma_start(wt[:, k], w1s[s, :, m, k])
                        else:
                            nc.scalar.dma_start(wt[:], w1s[s, :, m])
                        for ci, (c0, w) in enumerate(cols):
                            ps = psump.tile([P, NCHUNK], f32, tag="ps")
                            for k in range(KO1):
                                nc.tensor.matmul(
                                    ps[:, :w],
                                    wt[:, k],
                                    xcs[ci][:, k, :w],
                                    start=(k == 0),
                                    stop=(k == KO1 - 1),
                                )
                            ho = hopool.tile([P, NCHUNK], bf16, tag="ho")
                            nc.scalar.activation(
                                ho[:, :w], ps[:, :w], Silu, bias=b1_sb[:, s, m : m + 1]
                            )
                            nc.gpsimd.dma_start(h_dram[:, m, c0 : c0 + w], ho[:, :w])
                    hs.append(h_dram)

            # ---- layer 2: y = W2.T @ h, per expert slot ----
            # Slot-1 W2 loads into the space the FFN1 pools just freed,
            # overlapping slot-0 FFN2 compute.
            with tc.tile_pool(name="w2b", bufs=1) as w2bpool:
                w2sb1 = w2bpool.tile([P, KO2, D], bf16, tag="w2b", name="w2sb1")
                nc.sync.dma_start(w2sb1[:], w2s[1])
                for s in range(S):
                    w2sb = w2sb0 if s == 0 else w2sb1
                    cols = _chunks(caps[s], NCHUNK2)
                    for c0, w in cols:
                        hi = hipool.tile([P, KO2, NCHUNK2], bf16, tag="hi")
                        nc.sync.dma_start(hi[:, :, :w], hs[s][:, :, c0 : c0 + w])
                        for m2 in range(MO2):
                            ps = psump.tile([P, NCHUNK], f32, tag="ps")
                            for k2 in range(KO2):
                                nc.tensor.matmul(
                                    ps[:, :w],
                                    w2sb[:, k2, ts(m2, P)],
                                    hi[:, k2, :w],
                                    start=(k2 == 0),
                                    stop=(k2 == KO2 - 1),
                                )
                            yt = ypool.tile([P, NCHUNK2], f32, tag="yt")
                            nc.vector.tensor_copy(yt[:, :w], ps[:, :w])
                            nc.gpsimd.dma_start(
                                yT[:, m2, soff[s] + c0 : soff[s] + c0 + w],
                                yt[:, :w],
                            )

    nc.compile()
    _PROGRAM_CACHE[caps] = nc
    return nc


def kernel(x, w_router, b_router, w1, b1, w2, b2):
    _install_ntff_hook_shim()
    from concourse.bass_utils import run_bass_kernel_spmd

    x = np.asarray(x, dtype=np.float32)
    w_router = np.asarray(w_router, dtype=np.float32)
    b_router = np.asarray(b_router, dtype=np.float32)
    w1 = np.asarray(w1, dtype=np.float32)
    b1 = np.asarray(b1, dtype=np.float32)
    w2 = np.asarray(w2, dtype=np.float32)
    b2 = np.asarray(b2, dtype=np.float32)

    # ---- host router: top-2 + softmax over the selected logits ----
    logits = x @ w_router + b_router                    # [B, E]
    idx2 = np.argpartition(-logits, TOPK, axis=1)[:, :TOPK]
    vals = np.take_along_axis(logits, idx2, axis=1)
    order = np.argsort(-vals, axis=1)
    topk_i = np.take_along_axis(idx2, order, axis=1)    # [B, 2] descending
    topk_v = np.take_along_axis(vals, order, axis=1)
    topk_v = topk_v - topk_v.max(axis=1, keepdims=True)
    ew = np.exp(topk_v)
    cw = ew / ew.sum(axis=1, keepdims=True)             # [B, 2]

    # ---- dispatch: group (token, expert) pairs by expert ----
    eids = topk_i.ravel()                               # [B*2]
    toks = np.repeat(np.arange(B, dtype=np.int64), TOPK)
    wts = cw.ravel().astype(np.float32)
    perm = np.argsort(eids, kind="stable")
    toks_s, wts_s = toks[perm], wts[perm]
    counts = np.bincount(eids, minlength=E)
    offs = np.concatenate([[0], np.cumsum(counts)])

    # Rank experts by load: biggest 8 -> slot 0, smallest 8 -> slot 1.
    rank = np.argsort(-counts, kind="stable")           # expert ids, desc count
    # slot_expert[c][s] = expert id handled by core c, slot s
    slot_expert = [[int(rank[s * N_CORES + c]) for s in range(S)] for c in range(N_CORES)]
    caps = tuple(
        max(512, int(max(counts[rank[s * N_CORES + c]] for c in range(N_CORES))))
        for s in range(S)
    )
    CT = sum(caps)
    soff = [0]
    for c_ in caps:
        soff.append(soff[-1] + c_)

    nc = _build_program(caps)

    xTfull = np.ascontiguousarray(x.T)                  # [D, B] f32

    in_maps = []
    tok_lists = []
    for c in range(N_CORES):
        xcore = np.zeros((D, CT), dtype=BF16)
        core_toks = []
        for s in range(S):
            e = slot_expert[c][s]
            te = toks_s[offs[e] : offs[e + 1]]
            core_toks.append((te, wts_s[offs[e] : offs[e + 1]], e))
            xcore[:, soff[s] : soff[s] + len(te)] = xTfull[:, te].astype(BF16)
        tok_lists.append(core_toks)
        # [D, CT] -> [P, KO1, CT] with d = ko*P + p
        xcore = np.ascontiguousarray(xcore.reshape(KO1, P, CT).transpose(1, 0, 2))
        es = [slot_expert[c][s] for s in range(S)]
        w1c = np.ascontiguousarray(
            w1[es].astype(BF16).reshape(S, KO1, P, MO1, P).transpose(0, 2, 3, 1, 4)
        )  # [S, P, MO1, KO1, P]
        b1c = np.ascontiguousarray(
            b1[es].reshape(S, MO1, P).transpose(0, 2, 1)
        )  # [S, P, MO1]
        w2c = np.ascontiguousarray(
            w2[es].astype(BF16).reshape(S, KO2, P, D).transpose(0, 2, 1, 3)
        )  # [S, P, KO2, D]
        in_maps.append({"xT": xcore, "w1s": w1c, "b1s": b1c, "w2s": w2c})

    kw = {}
    if TRACE:
        kw = dict(trace=True)
        if TRACE_CORES is not None:
            kw["trace_cores"] = TRACE_CORES
    res = run_bass_kernel_spmd(nc, in_maps, core_ids=list(range(N_CORES)), **kw)
    global LAST_RESULTS
    LAST_RESULTS = res

    # ---- host combine: out = x + sum_e cw_e * (y_e + b2_e) ----
    out = x.copy()
    for c in range(N_CORES):
        yT = np.asarray(res.results[c]["yT"], dtype=np.float32)  # [P, MO2, CT]
        for s in range(S):
            te, we, e = tok_lists[c][s]
            n = len(te)
            if n == 0:
                continue
            y2 = yT[:, :, soff[s] : soff[s] + n]                 # [P, MO2, n]
            y2 = y2.transpose(1, 0, 2).reshape(D, n)             # d = m2*P + p
            out[te] += we[:, None] * (y2.T + b2[e])
    return out
